# revision 4
# baseline (speedup 1.0000x reference)
"""Trainium2 Bass kernel for nn_HardwareOptimizedSpikeProcessor.

Reference semantics (per timestep t):
    acc += (s_t @ (W*mask).T) * 2**scale_exp     # [B, Cout]
    spk  = acc >= 2**threshold_exp
    acc  = acc * (1 - spk)
    out[:, :, t] = spk

Key trick: feed the PE *prefix-summed* spikes p_t = cumsum_t(s) (ints
<= 128, exact in bf16). Then the matmul yields prefix sums
S_t = sum_{tau<=t} c_tau directly (exact fp32 ints < 2^22). The
LIF-with-reset recurrence becomes a threshold-crossing rule:

    spike at t  <=>  S_t >= Q,   and on spike  Q <- S_t + thr
    (Q holds S_{last spike} + thr; init Q = thr)

which is 2 dependent DVE ops per timestep (tensor_tensor is_ge +
copy_predicated) instead of 3, and emits the spike output directly.
The ACT engine evicts PSUM to SBUF twice per chunk half: S (bias 0)
for the compares and V = S + thr (bias thr) for the Q updates.

The scan runs as two independent half-lane chains, interleaved so
consecutive DVE ops have no read-after-write hazard.

Sharding: 2-way batch x 4-way Cout; matmul free dim = 32 samples x 16
timesteps; chunks [8, 8, 16x6, 8, 8] pipeline PE -> ACT evict -> DVE
scan, with small chunks at both ends (early PE start, short scan
tail) and 4-step eviction pieces on the tail chunk. Weights ship as
int8 (halves the startup DMA front-load) and are upconverted to bf16
on the otherwise-idle ACT engine. All arithmetic is exact, so the
result is bit-identical to the fp32 reference.
"""

import sys

for _p in ("/opt/trn_rl_repo",):
    if _p not in sys.path:
        sys.path.insert(0, _p)

import numpy as np
import ml_dtypes

import concourse.bass as bass
import concourse.mybir as mybir
import concourse.tile as tile
from concourse.bass_utils import run_bass_kernel_spmd

B, CIN, COUT, T = 64, 2048, 2048, 128
NCORES = 8
BS = 2                      # batch shard
CS = 4                      # cout shard
BLOC = B // BS              # 32 samples per core
MLOC = 4                    # cout 128-tiles per core (512 channels)
KC = CIN // 128             # 16 contraction chunks

CHUNKS = [8, 8] + [16] * 6 + [8, 8]
assert sum(CHUNKS) == T
CTMAX = max(CHUNKS)
HMAX = 8                    # half-chunk (eviction granularity)

_MAX_WAITS = 1


def _split_excess_waits(nc):
    """This container's walrus build accepts at most one sync-wait per
    instruction; spill extra waits onto same-engine NOPs placed before the
    offending instruction."""
    for f in nc.m.functions:
        for bb in f.blocks:
            new_list = []
            for ins in bb.instructions:
                si = ins.sync_info
                waits = list(si.on_wait) if si is not None and si.on_wait else []
                if len(waits) > _MAX_WAITS:
                    extra, keep = waits[:-_MAX_WAITS], waits[-_MAX_WAITS:]
                    for i in range(0, len(extra), _MAX_WAITS):
                        nop = mybir.InstNoOp(
                            name=f"{ins.name}-waitsplit-{i}", ins=[], outs=[]
                        )
                        nop.engine = ins.engine
                        nop.sync_info = mybir.SyncInfo(
                            on_wait=extra[i : i + _MAX_WAITS], on_update=[]
                        )
                        new_list.append(nop)
                    ins.sync_info = mybir.SyncInfo(
                        on_wait=keep,
                        on_update=list(si.on_update) if si.on_update else [],
                    )
                new_list.append(ins)
            bb.instructions[:] = new_list


def _build(thr: float):
    f32 = mybir.dt.float32
    bf16 = mybir.dt.bfloat16
    u8 = mybir.dt.uint8
    nc = bass.Bass()

    i8 = mybir.dt.int8
    # weights (scale folded in), shipped as int8 to halve the startup
    # DMA front-load: [m, cin_lo, k, cout_lo]
    wt_d = nc.dram_tensor("wt", [MLOC, 128, KC, 128], i8, kind="ExternalInput")
    # prefix-summed spikes, one contiguous tensor per chunk: [cl, k, t, b]
    spk_ds = [
        nc.dram_tensor(f"spk{j}", [128, KC, ct * BLOC], bf16, kind="ExternalInput")
        for j, ct in enumerate(CHUNKS)
    ]
    # spikes out: [cout_lo, t, m*32+b]
    out_d = nc.dram_tensor("out", [128, T, 128], u8, kind="ExternalOutput")

    with tile.TileContext(nc) as tc:
        with (
            tc.tile_pool(name="const", bufs=1) as const,
            tc.tile_pool(name="spool", bufs=3) as spool,
            tc.tile_pool(name="cpool", bufs=4) as cpool,
            tc.tile_pool(name="opool", bufs=2) as opool,
            tc.tile_pool(name="psum", bufs=2, space="PSUM") as psum,
        ):
            wt8_sb = const.tile([128, MLOC, KC, 128], i8)
            wt_sb = const.tile([128, MLOC, KC, 128], bf16)
            q = const.tile([128, 128], f32)
            nc.vector.memset(q[:], thr)

            # DMA order: int8 weights m0/m1, spike chunk 0, weights m2/m3,
            # remaining spike chunks. The ACT engine (idle until the first
            # eviction ~23us in) upconverts int8 -> bf16 as each m lands.
            spk_sbs = []
            for j, ct in enumerate(CHUNKS):
                spk_sbs.append(
                    spool.tile([128, KC, CTMAX * BLOC], bf16, tag="spk", name=f"spk{j}")
                )
            nc.sync.dma_start(wt8_sb[:, 0], wt_d[0])
            nc.sync.dma_start(wt8_sb[:, 1], wt_d[1])
            nc.sync.dma_start(
                spk_sbs[0][:, :, : CHUNKS[0] * BLOC], spk_ds[0][:]
            )
            for m in range(2, MLOC):
                nc.sync.dma_start(wt8_sb[:, m], wt_d[m])
            for j in range(1, len(CHUNKS)):
                ct = CHUNKS[j]
                nc.sync.dma_start(
                    spk_sbs[j][:, :, : ct * BLOC], spk_ds[j][:]
                )
            for m in range(MLOC):
                nc.scalar.activation(
                    wt_sb[:, m], wt8_sb[:, m], mybir.ActivationFunctionType.Copy
                )

            t0 = 0
            for j, ct in enumerate(CHUNKS):
                nf = ct * BLOC
                ps = psum.tile([128, MLOC, 512], f32, tag="ps", name="ps")
                for m in range(MLOC):
                    for k in range(KC):
                        nc.tensor.matmul(
                            ps[:, m, :nf],
                            lhsT=wt_sb[:, m, k, :],
                            rhs=spk_sbs[j][:, k, :nf],
                            start=(k == 0),
                            stop=(k == KC - 1),
                        )
                ob = opool.tile([128, CTMAX, 128], u8, tag="ob")
                last = j == len(CHUNKS) - 1
                # halves: eviction+scan granularity (4t pieces on the tail
                # chunk so the first compare starts ~0.6us after the last
                # matmul instead of ~2.2us)
                step = 4 if last else HMAX
                for ta in range(0, ct, step):
                    hct = min(step, ct - ta)
                    cbs = cpool.tile([128, HMAX, MLOC, BLOC], f32, tag="cbs")
                    cbv = cpool.tile([128, HMAX, MLOC, BLOC], f32, tag="cbv")
                    src = ps[:, :, ta * BLOC : (ta + hct) * BLOC].rearrange(
                        "p m (t b) -> p m t b", t=hct
                    )
                    nc.scalar.activation(
                        cbs[:, :hct].rearrange("p t m b -> p m t b"),
                        src,
                        mybir.ActivationFunctionType.Copy,
                    )
                    nc.scalar.activation(
                        cbv[:, :hct].rearrange("p t m b -> p m t b"),
                        src,
                        mybir.ActivationFunctionType.Copy,
                        bias=thr,
                    )
                    # two independent half-lane chains (cols 0:64 / 64:128)
                    # interleaved so consecutive DVE ops have no RAW hazard
                    for t in range(hct):
                        nc.vector.tensor_tensor(
                            ob[:, ta + t, 0:64],
                            cbs[:, t, 0:2],
                            q[:, 0:64],
                            mybir.AluOpType.is_ge,
                        )
                        nc.vector.tensor_tensor(
                            ob[:, ta + t, 64:128],
                            cbs[:, t, 2:4],
                            q[:, 64:128],
                            mybir.AluOpType.is_ge,
                        )
                        if last and ta + t == ct - 1:
                            continue  # Q is dead after the final timestep
                        nc.vector.copy_predicated(
                            q[:, 0:64], ob[:, ta + t, 0:64], cbv[:, t, 0:2]
                        )
                        nc.vector.copy_predicated(
                            q[:, 64:128], ob[:, ta + t, 64:128], cbv[:, t, 2:4]
                        )
                    if last:
                        # ship each finished piece immediately
                        nc.sync.dma_start(
                            out_d[:, t0 + ta : t0 + ta + hct, :],
                            ob[:, ta : ta + hct],
                        )
                if not last:
                    nc.sync.dma_start(out_d[:, t0 : t0 + ct, :], ob[:, :ct])
                t0 += ct

    _split_excess_waits(nc)
    return nc


def _prep_inputs(spikes, weights, mask, scale_exp):
    wm = weights * mask  # integers <= 127, exact
    scale = np.exp2(scale_exp.astype(np.float64)).astype(np.float32)
    wm = (wm * scale[:, None]).astype(np.float32)
    # per cout-quarter: [512, 2048] -> [m, cin_lo, k, cout_lo]
    wts = []
    for cs in range(CS):
        a = wm[cs * 512 : (cs + 1) * 512]
        a = a.reshape(MLOC, 128, KC, 128).transpose(0, 3, 2, 1)
        wts.append(np.ascontiguousarray(a).astype(np.int8))
    # prefix-sum over t, then per batch-half: [32, 2048, 128] -> chunked
    # [cin_lo, k, t, b] contiguous per chunk
    p = np.cumsum(spikes, axis=2, dtype=np.float32)  # ints <= 128, exact
    spks = []
    for bs in range(BS):
        s = p[bs * BLOC : (bs + 1) * BLOC]
        a = s.reshape(BLOC, KC, 128, T).transpose(2, 1, 3, 0)  # [cl, k, t, b]
        a = a.astype(ml_dtypes.bfloat16)
        blks = {}
        t0 = 0
        for j, ct in enumerate(CHUNKS):
            blks[f"spk{j}"] = np.ascontiguousarray(
                a[:, :, t0 : t0 + ct, :]
            ).reshape(128, KC, ct * BLOC)
            t0 += ct
        spks.append(blks)
    return wts, spks


_CACHE = {}


def _get_program(thr: float):
    if thr not in _CACHE:
        _CACHE[thr] = _build(thr)
    return _CACHE[thr]


def kernel(spikes, weights, mask, scale_exp, threshold_exp, **run_kwargs):
    thr = float(2.0 ** int(np.asarray(threshold_exp)))
    nc = _get_program(thr)
    wts, spks = _prep_inputs(
        np.asarray(spikes, dtype=np.float32),
        np.asarray(weights, dtype=np.float32),
        np.asarray(mask, dtype=np.float32),
        np.asarray(scale_exp),
    )
    # core i = (bs, cs): bs = i // CS, cs = i % CS
    in_maps = [
        {"wt": wts[i % CS], **spks[i // CS]} for i in range(NCORES)
    ]
    res = run_bass_kernel_spmd(
        nc, in_maps, core_ids=list(range(NCORES)), **run_kwargs
    )
    full = np.empty((B, COUT, T), dtype=np.float32)
    for i in range(NCORES):
        bs, cs = i // CS, i % CS
        a = np.asarray(res.results[i]["out"])  # [cout_lo, t, m*32+b] spikes
        a = a.reshape(128, T, MLOC, BLOC)
        sp = a.transpose(3, 2, 0, 1).reshape(BLOC, 512, T)
        full[bs * BLOC : (bs + 1) * BLOC, cs * 512 : (cs + 1) * 512] = sp
    if run_kwargs:
        return full, res
    return full


# revision 5
# speedup vs baseline: 1.0104x; 1.0104x over previous
"""Trainium2 Bass kernel for nn_HardwareOptimizedSpikeProcessor.

Reference semantics (per timestep t):
    acc += (s_t @ (W*mask).T) * 2**scale_exp     # [B, Cout]
    spk  = acc >= 2**threshold_exp
    acc  = acc * (1 - spk)
    out[:, :, t] = spk

Key trick: feed the PE *prefix-summed* spikes p_t = cumsum_t(s) (ints
<= 128, exact in bf16). Then the matmul yields prefix sums
S_t = sum_{tau<=t} c_tau directly (exact fp32 ints < 2^22). The
LIF-with-reset recurrence becomes a threshold-crossing rule:

    spike at t  <=>  S_t >= Q,   and on spike  Q <- S_t + thr
    (Q holds S_{last spike} + thr; init Q = thr)

which is 2 dependent DVE ops per timestep (tensor_tensor is_ge +
copy_predicated) instead of 3, and emits the spike output directly.
The ACT engine evicts PSUM to SBUF twice per chunk half: S (bias 0)
for the compares and V = S + thr (bias thr) for the Q updates.

The scan runs as two independent half-lane chains, interleaved so
consecutive DVE ops have no read-after-write hazard.

Sharding: 2-way batch x 4-way Cout; matmul free dim = 32 samples x 16
timesteps; chunks [8, 8, 16x6, 8, 8] pipeline PE -> ACT evict -> DVE
scan, with small chunks at both ends (early PE start, short scan
tail) and 4-step eviction pieces on the tail chunk. Weights ship as
int8 (upconverted to bf16 on the otherwise-idle ACT engine) and the
prefix-summed spikes ship as fp8e4: chunk-LOCAL cumsum keeps values
<= 16 (exact in fp8e4m3), the PE accepts mixed bf16 x fp8 operands at
full rate (hardware-verified), and Q is re-based by one DVE subtract
per chunk boundary. This halves both the startup front-load and the
sustained spike DMA. All arithmetic is exact, so the result is
bit-identical to the fp32 reference.
"""

import sys

for _p in ("/opt/trn_rl_repo",):
    if _p not in sys.path:
        sys.path.insert(0, _p)

import numpy as np
import ml_dtypes

import concourse.bass as bass
import concourse.mybir as mybir
import concourse.tile as tile
from concourse.bass_utils import run_bass_kernel_spmd

B, CIN, COUT, T = 64, 2048, 2048, 128
NCORES = 8
BS = 2                      # batch shard
CS = 4                      # cout shard
BLOC = B // BS              # 32 samples per core
MLOC = 4                    # cout 128-tiles per core (512 channels)
KC = CIN // 128             # 16 contraction chunks

CHUNKS = [8, 8] + [16] * 6 + [8, 8]
assert sum(CHUNKS) == T
CTMAX = max(CHUNKS)
HMAX = 8                    # half-chunk (eviction granularity)

_MAX_WAITS = 1


def _split_excess_waits(nc):
    """This container's walrus build accepts at most one sync-wait per
    instruction; spill extra waits onto same-engine NOPs placed before the
    offending instruction."""
    for f in nc.m.functions:
        for bb in f.blocks:
            new_list = []
            for ins in bb.instructions:
                si = ins.sync_info
                waits = list(si.on_wait) if si is not None and si.on_wait else []
                if len(waits) > _MAX_WAITS:
                    extra, keep = waits[:-_MAX_WAITS], waits[-_MAX_WAITS:]
                    for i in range(0, len(extra), _MAX_WAITS):
                        nop = mybir.InstNoOp(
                            name=f"{ins.name}-waitsplit-{i}", ins=[], outs=[]
                        )
                        nop.engine = ins.engine
                        nop.sync_info = mybir.SyncInfo(
                            on_wait=extra[i : i + _MAX_WAITS], on_update=[]
                        )
                        new_list.append(nop)
                    ins.sync_info = mybir.SyncInfo(
                        on_wait=keep,
                        on_update=list(si.on_update) if si.on_update else [],
                    )
                new_list.append(ins)
            bb.instructions[:] = new_list


def _build(thr: float):
    f32 = mybir.dt.float32
    bf16 = mybir.dt.bfloat16
    u8 = mybir.dt.uint8
    nc = bass.Bass()

    i8 = mybir.dt.int8
    # weights (scale folded in), shipped as int8 to halve the startup
    # DMA front-load: [m, cin_lo, k, cout_lo]
    wt_d = nc.dram_tensor("wt", [MLOC, 128, KC, 128], i8, kind="ExternalInput")
    fp8 = mybir.dt.float8e4
    # chunk-locally prefix-summed spikes (values <= 16, exact in fp8e4),
    # one contiguous tensor per chunk: [cl, k, t, b]
    spk_ds = [
        nc.dram_tensor(f"spk{j}", [128, KC, ct * BLOC], fp8, kind="ExternalInput")
        for j, ct in enumerate(CHUNKS)
    ]
    # spikes out: [cout_lo, t, m*32+b]
    out_d = nc.dram_tensor("out", [128, T, 128], u8, kind="ExternalOutput")

    with tile.TileContext(nc) as tc:
        with (
            tc.tile_pool(name="const", bufs=1) as const,
            tc.tile_pool(name="spool", bufs=3) as spool,
            tc.tile_pool(name="cpool", bufs=4) as cpool,
            tc.tile_pool(name="opool", bufs=2) as opool,
            tc.tile_pool(name="psum", bufs=2, space="PSUM") as psum,
        ):
            wt8_sb = const.tile([128, MLOC, KC, 128], i8)
            wt_sb = const.tile([128, MLOC, KC, 128], bf16)
            q = const.tile([128, 128], f32)
            nc.vector.memset(q[:], thr)

            # DMA order: int8 weights m0/m1, spike chunk 0, weights m2/m3,
            # remaining spike chunks. The ACT engine (idle until the first
            # eviction ~23us in) upconverts int8 -> bf16 as each m lands.
            spk_sbs = []
            for j, ct in enumerate(CHUNKS):
                spk_sbs.append(
                    spool.tile([128, KC, CTMAX * BLOC], fp8, tag="spk", name=f"spk{j}")
                )
            nc.sync.dma_start(wt8_sb[:, 0], wt_d[0])
            nc.sync.dma_start(wt8_sb[:, 1], wt_d[1])
            nc.sync.dma_start(
                spk_sbs[0][:, :, : CHUNKS[0] * BLOC], spk_ds[0][:]
            )
            for m in range(2, MLOC):
                nc.sync.dma_start(wt8_sb[:, m], wt_d[m])
            for j in range(1, len(CHUNKS)):
                ct = CHUNKS[j]
                nc.sync.dma_start(
                    spk_sbs[j][:, :, : ct * BLOC], spk_ds[j][:]
                )
            for m in range(MLOC):
                nc.scalar.activation(
                    wt_sb[:, m], wt8_sb[:, m], mybir.ActivationFunctionType.Copy
                )

            t0 = 0
            for j, ct in enumerate(CHUNKS):
                nf = ct * BLOC
                ps = psum.tile([128, MLOC, 512], f32, tag="ps", name="ps")
                for m in range(MLOC):
                    for k in range(KC):
                        nc.tensor.matmul(
                            ps[:, m, :nf],
                            lhsT=wt_sb[:, m, k, :],
                            rhs=spk_sbs[j][:, k, :nf],
                            start=(k == 0),
                            stop=(k == KC - 1),
                        )
                ob = opool.tile([128, CTMAX, 128], u8, tag="ob")
                last = j == len(CHUNKS) - 1
                if j > 0:
                    # re-base Q into this chunk's local prefix coordinates
                    nc.vector.tensor_tensor(
                        q[:], q[:], prev_s_last, mybir.AluOpType.subtract
                    )
                # halves: eviction+scan granularity (4t pieces on the tail
                # chunk so the first compare starts ~0.6us after the last
                # matmul instead of ~2.2us)
                step = 4 if last else HMAX
                for ta in range(0, ct, step):
                    hct = min(step, ct - ta)
                    cbs = cpool.tile([128, HMAX, MLOC, BLOC], f32, tag="cbs")
                    cbv = cpool.tile([128, HMAX, MLOC, BLOC], f32, tag="cbv")
                    src = ps[:, :, ta * BLOC : (ta + hct) * BLOC].rearrange(
                        "p m (t b) -> p m t b", t=hct
                    )
                    nc.scalar.activation(
                        cbs[:, :hct].rearrange("p t m b -> p m t b"),
                        src,
                        mybir.ActivationFunctionType.Copy,
                    )
                    nc.scalar.activation(
                        cbv[:, :hct].rearrange("p t m b -> p m t b"),
                        src,
                        mybir.ActivationFunctionType.Copy,
                        bias=thr,
                    )
                    # two independent half-lane chains (cols 0:64 / 64:128)
                    # interleaved so consecutive DVE ops have no RAW hazard
                    for t in range(hct):
                        nc.vector.tensor_tensor(
                            ob[:, ta + t, 0:64],
                            cbs[:, t, 0:2],
                            q[:, 0:64],
                            mybir.AluOpType.is_ge,
                        )
                        nc.vector.tensor_tensor(
                            ob[:, ta + t, 64:128],
                            cbs[:, t, 2:4],
                            q[:, 64:128],
                            mybir.AluOpType.is_ge,
                        )
                        if last and ta + t == ct - 1:
                            continue  # Q is dead after the final timestep
                        nc.vector.copy_predicated(
                            q[:, 0:64], ob[:, ta + t, 0:64], cbv[:, t, 0:2]
                        )
                        nc.vector.copy_predicated(
                            q[:, 64:128], ob[:, ta + t, 64:128], cbv[:, t, 2:4]
                        )
                    if ta + hct == ct:
                        prev_s_last = cbs[:, hct - 1]
                    if last:
                        # ship each finished piece immediately
                        nc.sync.dma_start(
                            out_d[:, t0 + ta : t0 + ta + hct, :],
                            ob[:, ta : ta + hct],
                        )
                if not last:
                    nc.sync.dma_start(out_d[:, t0 : t0 + ct, :], ob[:, :ct])
                t0 += ct

    _split_excess_waits(nc)
    return nc


def _prep_inputs(spikes, weights, mask, scale_exp):
    wm = weights * mask  # integers <= 127, exact
    scale = np.exp2(scale_exp.astype(np.float64)).astype(np.float32)
    wm = (wm * scale[:, None]).astype(np.float32)
    # per cout-quarter: [512, 2048] -> [m, cin_lo, k, cout_lo]
    wts = []
    for cs in range(CS):
        a = wm[cs * 512 : (cs + 1) * 512]
        a = a.reshape(MLOC, 128, KC, 128).transpose(0, 3, 2, 1)
        wts.append(np.ascontiguousarray(a).astype(np.int8))
    # chunk-local prefix-sum over t (values <= 16, exact in fp8e4), per
    # batch-half: [32, 2048, 128] -> chunked [cin_lo, k, t, b]
    p = np.cumsum(spikes, axis=2, dtype=np.float32)
    spks = []
    for bs in range(BS):
        s = p[bs * BLOC : (bs + 1) * BLOC]
        a = s.reshape(BLOC, KC, 128, T).transpose(2, 1, 3, 0)  # [cl, k, t, b]
        blks = {}
        t0 = 0
        for j, ct in enumerate(CHUNKS):
            loc = a[:, :, t0 : t0 + ct, :]
            if t0 > 0:
                loc = loc - a[:, :, t0 - 1 : t0, :]
            blks[f"spk{j}"] = np.ascontiguousarray(
                loc.astype(ml_dtypes.float8_e4m3)
            ).reshape(128, KC, ct * BLOC)
            t0 += ct
        spks.append(blks)
    return wts, spks


_CACHE = {}


def _get_program(thr: float):
    if thr not in _CACHE:
        _CACHE[thr] = _build(thr)
    return _CACHE[thr]


def kernel(spikes, weights, mask, scale_exp, threshold_exp, **run_kwargs):
    thr = float(2.0 ** int(np.asarray(threshold_exp)))
    nc = _get_program(thr)
    wts, spks = _prep_inputs(
        np.asarray(spikes, dtype=np.float32),
        np.asarray(weights, dtype=np.float32),
        np.asarray(mask, dtype=np.float32),
        np.asarray(scale_exp),
    )
    # core i = (bs, cs): bs = i // CS, cs = i % CS
    in_maps = [
        {"wt": wts[i % CS], **spks[i // CS]} for i in range(NCORES)
    ]
    res = run_bass_kernel_spmd(
        nc, in_maps, core_ids=list(range(NCORES)), **run_kwargs
    )
    full = np.empty((B, COUT, T), dtype=np.float32)
    for i in range(NCORES):
        bs, cs = i // CS, i % CS
        a = np.asarray(res.results[i]["out"])  # [cout_lo, t, m*32+b] spikes
        a = a.reshape(128, T, MLOC, BLOC)
        sp = a.transpose(3, 2, 0, 1).reshape(BLOC, 512, T)
        full[bs * BLOC : (bs + 1) * BLOC, cs * 512 : (cs + 1) * 512] = sp
    if run_kwargs:
        return full, res
    return full


# revision 6
# speedup vs baseline: 1.0114x; 1.0009x over previous
"""Trainium2 Bass kernel for nn_HardwareOptimizedSpikeProcessor.

Reference semantics (per timestep t):
    acc += (s_t @ (W*mask).T) * 2**scale_exp     # [B, Cout]
    spk  = acc >= 2**threshold_exp
    acc  = acc * (1 - spk)
    out[:, :, t] = spk

Key trick: feed the PE *prefix-summed* spikes p_t = cumsum_t(s) (ints
<= 128, exact in bf16). Then the matmul yields prefix sums
S_t = sum_{tau<=t} c_tau directly (exact fp32 ints < 2^22). The
LIF-with-reset recurrence becomes a threshold-crossing rule:

    spike at t  <=>  S_t >= Q,   and on spike  Q <- S_t + thr
    (Q holds S_{last spike} + thr; init Q = thr)

which is 2 dependent DVE ops per timestep (tensor_tensor is_ge +
copy_predicated) instead of 3, and emits the spike output directly.
The ACT engine evicts PSUM to SBUF twice per chunk half: S (bias 0)
for the compares and V = S + thr (bias thr) for the Q updates.

The scan runs as two independent half-lane chains, interleaved so
consecutive DVE ops have no read-after-write hazard.

Sharding: 2-way batch x 4-way Cout; matmul free dim = 32 samples x 16
timesteps; chunks [8, 8, 16x6, 8, 8] pipeline PE -> ACT evict -> DVE
scan, with small chunks at both ends (early PE start, short scan
tail) and 4-step eviction pieces on the tail chunk. Weights ship as
int8 (upconverted to bf16 on the otherwise-idle ACT engine) and the
prefix-summed spikes ship as fp8e4: chunk-LOCAL cumsum keeps values
<= 16 (exact in fp8e4m3), the PE accepts mixed bf16 x fp8 operands at
full rate (hardware-verified), and Q is re-based by one DVE subtract
per chunk boundary. This halves both the startup front-load and the
sustained spike DMA. All arithmetic is exact, so the result is
bit-identical to the fp32 reference.
"""

import sys

for _p in ("/opt/trn_rl_repo",):
    if _p not in sys.path:
        sys.path.insert(0, _p)

import numpy as np
import ml_dtypes

import concourse.bass as bass
import concourse.mybir as mybir
import concourse.tile as tile
from concourse.bass_utils import run_bass_kernel_spmd

B, CIN, COUT, T = 64, 2048, 2048, 128
NCORES = 8
BS = 2                      # batch shard
CS = 4                      # cout shard
BLOC = B // BS              # 32 samples per core
MLOC = 4                    # cout 128-tiles per core (512 channels)
KC = CIN // 128             # 16 contraction chunks

CHUNKS = [8, 8] + [16] * 6 + [8, 8]
assert sum(CHUNKS) == T
CTMAX = max(CHUNKS)
HMAX = 8                    # half-chunk (eviction granularity)

_MAX_WAITS = 1


def _split_excess_waits(nc):
    """This container's walrus build accepts at most one sync-wait per
    instruction; spill extra waits onto same-engine NOPs placed before the
    offending instruction."""
    for f in nc.m.functions:
        for bb in f.blocks:
            new_list = []
            for ins in bb.instructions:
                si = ins.sync_info
                waits = list(si.on_wait) if si is not None and si.on_wait else []
                if len(waits) > _MAX_WAITS:
                    extra, keep = waits[:-_MAX_WAITS], waits[-_MAX_WAITS:]
                    for i in range(0, len(extra), _MAX_WAITS):
                        nop = mybir.InstNoOp(
                            name=f"{ins.name}-waitsplit-{i}", ins=[], outs=[]
                        )
                        nop.engine = ins.engine
                        nop.sync_info = mybir.SyncInfo(
                            on_wait=extra[i : i + _MAX_WAITS], on_update=[]
                        )
                        new_list.append(nop)
                    ins.sync_info = mybir.SyncInfo(
                        on_wait=keep,
                        on_update=list(si.on_update) if si.on_update else [],
                    )
                new_list.append(ins)
            bb.instructions[:] = new_list


def _build(thr: float):
    f32 = mybir.dt.float32
    bf16 = mybir.dt.bfloat16
    u8 = mybir.dt.uint8
    nc = bass.Bass()

    i8 = mybir.dt.int8
    # weights (scale folded in), shipped as int8 to halve the startup
    # DMA front-load: [m, cin_lo, k, cout_lo]
    wt_d = nc.dram_tensor("wt", [MLOC, 128, KC, 128], i8, kind="ExternalInput")
    fp8 = mybir.dt.float8e4
    # chunk-locally prefix-summed spikes (values <= 16, exact in fp8e4),
    # one contiguous tensor per chunk: [cl, k, t, b]
    spk_ds = [
        nc.dram_tensor(f"spk{j}", [128, KC, ct * BLOC], fp8, kind="ExternalInput")
        for j, ct in enumerate(CHUNKS)
    ]
    # spikes out: [cout_lo, t, m*32+b]
    out_d = nc.dram_tensor("out", [128, T, 128], u8, kind="ExternalOutput")

    with tile.TileContext(nc) as tc:
        with (
            tc.tile_pool(name="const", bufs=1) as const,
            tc.tile_pool(name="spool", bufs=3) as spool,
            tc.tile_pool(name="cpool", bufs=4) as cpool,
            tc.tile_pool(name="opool", bufs=2) as opool,
            tc.tile_pool(name="psum", bufs=2, space="PSUM") as psum,
        ):
            wt8_sb = const.tile([128, MLOC, KC, 128], i8)
            wt_sb = const.tile([128, MLOC, KC, 128], bf16)
            q = const.tile([128, 128], f32)
            nc.vector.memset(q[:], thr)

            # DMA order: int8 weights m0/m1, spike chunk 0, weights m2/m3,
            # remaining spike chunks. The ACT engine (idle until the first
            # eviction ~23us in) upconverts int8 -> bf16 as each m lands.
            spk_sbs = []
            for j, ct in enumerate(CHUNKS):
                spk_sbs.append(
                    spool.tile([128, KC, CTMAX * BLOC], fp8, tag="spk", name=f"spk{j}")
                )
            nc.sync.dma_start(wt8_sb[:, 0], wt_d[0])
            nc.sync.dma_start(wt8_sb[:, 1], wt_d[1])
            nc.sync.dma_start(
                spk_sbs[0][:, :, : CHUNKS[0] * BLOC], spk_ds[0][:]
            )
            for m in range(2, MLOC):
                nc.sync.dma_start(wt8_sb[:, m], wt_d[m])
            for j in range(1, len(CHUNKS)):
                ct = CHUNKS[j]
                nc.sync.dma_start(
                    spk_sbs[j][:, :, : ct * BLOC], spk_ds[j][:]
                )
            for m in range(MLOC):
                nc.scalar.activation(
                    wt_sb[:, m], wt8_sb[:, m], mybir.ActivationFunctionType.Copy
                )

            t0 = 0
            for j, ct in enumerate(CHUNKS):
                nf = ct * BLOC
                ps = psum.tile([128, MLOC, 512], f32, tag="ps", name="ps")
                for m in range(MLOC):
                    for k in range(KC):
                        nc.tensor.matmul(
                            ps[:, m, :nf],
                            lhsT=wt_sb[:, m, k, :],
                            rhs=spk_sbs[j][:, k, :nf],
                            start=(k == 0),
                            stop=(k == KC - 1),
                        )
                ob = opool.tile([128, CTMAX, 128], u8, tag="ob")
                last = j == len(CHUNKS) - 1
                if j > 0:
                    # re-base Q into this chunk's local prefix coordinates
                    nc.vector.tensor_tensor(
                        q[:], q[:], prev_s_last, mybir.AluOpType.subtract
                    )
                # halves: eviction+scan granularity. On the tail chunk the
                # first piece is only 2 steps so the S+V evictions gating the
                # first compare finish ~0.8us after the last matmul; the
                # later 3-step pieces pipeline ahead of the scan.
                pieces = (
                    [(0, 2), (2, 3), (5, 3)]
                    if last
                    else [(ta, min(HMAX, ct - ta)) for ta in range(0, ct, HMAX)]
                )
                for ta, hct in pieces:
                    cbs = cpool.tile([128, HMAX, MLOC, BLOC], f32, tag="cbs")
                    cbv = cpool.tile([128, HMAX, MLOC, BLOC], f32, tag="cbv")
                    src = ps[:, :, ta * BLOC : (ta + hct) * BLOC].rearrange(
                        "p m (t b) -> p m t b", t=hct
                    )
                    nc.scalar.activation(
                        cbs[:, :hct].rearrange("p t m b -> p m t b"),
                        src,
                        mybir.ActivationFunctionType.Copy,
                    )
                    nc.scalar.activation(
                        cbv[:, :hct].rearrange("p t m b -> p m t b"),
                        src,
                        mybir.ActivationFunctionType.Copy,
                        bias=thr,
                    )
                    # two independent half-lane chains (cols 0:64 / 64:128)
                    # interleaved so consecutive DVE ops have no RAW hazard
                    for t in range(hct):
                        nc.vector.tensor_tensor(
                            ob[:, ta + t, 0:64],
                            cbs[:, t, 0:2],
                            q[:, 0:64],
                            mybir.AluOpType.is_ge,
                        )
                        nc.vector.tensor_tensor(
                            ob[:, ta + t, 64:128],
                            cbs[:, t, 2:4],
                            q[:, 64:128],
                            mybir.AluOpType.is_ge,
                        )
                        if last and ta + t == ct - 1:
                            continue  # Q is dead after the final timestep
                        nc.vector.copy_predicated(
                            q[:, 0:64], ob[:, ta + t, 0:64], cbv[:, t, 0:2]
                        )
                        nc.vector.copy_predicated(
                            q[:, 64:128], ob[:, ta + t, 64:128], cbv[:, t, 2:4]
                        )
                    if ta + hct == ct:
                        prev_s_last = cbs[:, hct - 1]
                    if last:
                        # ship each finished piece immediately
                        nc.sync.dma_start(
                            out_d[:, t0 + ta : t0 + ta + hct, :],
                            ob[:, ta : ta + hct],
                        )
                if not last:
                    nc.sync.dma_start(out_d[:, t0 : t0 + ct, :], ob[:, :ct])
                t0 += ct

    _split_excess_waits(nc)
    return nc


def _prep_inputs(spikes, weights, mask, scale_exp):
    wm = weights * mask  # integers <= 127, exact
    scale = np.exp2(scale_exp.astype(np.float64)).astype(np.float32)
    wm = (wm * scale[:, None]).astype(np.float32)
    # per cout-quarter: [512, 2048] -> [m, cin_lo, k, cout_lo]
    wts = []
    for cs in range(CS):
        a = wm[cs * 512 : (cs + 1) * 512]
        a = a.reshape(MLOC, 128, KC, 128).transpose(0, 3, 2, 1)
        wts.append(np.ascontiguousarray(a).astype(np.int8))
    # chunk-local prefix-sum over t (values <= 16, exact in fp8e4), per
    # batch-half: [32, 2048, 128] -> chunked [cin_lo, k, t, b]
    p = np.cumsum(spikes, axis=2, dtype=np.float32)
    spks = []
    for bs in range(BS):
        s = p[bs * BLOC : (bs + 1) * BLOC]
        a = s.reshape(BLOC, KC, 128, T).transpose(2, 1, 3, 0)  # [cl, k, t, b]
        blks = {}
        t0 = 0
        for j, ct in enumerate(CHUNKS):
            loc = a[:, :, t0 : t0 + ct, :]
            if t0 > 0:
                loc = loc - a[:, :, t0 - 1 : t0, :]
            blks[f"spk{j}"] = np.ascontiguousarray(
                loc.astype(ml_dtypes.float8_e4m3)
            ).reshape(128, KC, ct * BLOC)
            t0 += ct
        spks.append(blks)
    return wts, spks


_CACHE = {}


def _get_program(thr: float):
    if thr not in _CACHE:
        _CACHE[thr] = _build(thr)
    return _CACHE[thr]


def kernel(spikes, weights, mask, scale_exp, threshold_exp, **run_kwargs):
    thr = float(2.0 ** int(np.asarray(threshold_exp)))
    nc = _get_program(thr)
    wts, spks = _prep_inputs(
        np.asarray(spikes, dtype=np.float32),
        np.asarray(weights, dtype=np.float32),
        np.asarray(mask, dtype=np.float32),
        np.asarray(scale_exp),
    )
    # core i = (bs, cs): bs = i // CS, cs = i % CS
    in_maps = [
        {"wt": wts[i % CS], **spks[i // CS]} for i in range(NCORES)
    ]
    res = run_bass_kernel_spmd(
        nc, in_maps, core_ids=list(range(NCORES)), **run_kwargs
    )
    full = np.empty((B, COUT, T), dtype=np.float32)
    for i in range(NCORES):
        bs, cs = i // CS, i % CS
        a = np.asarray(res.results[i]["out"])  # [cout_lo, t, m*32+b] spikes
        a = a.reshape(128, T, MLOC, BLOC)
        sp = a.transpose(3, 2, 0, 1).reshape(BLOC, 512, T)
        full[bs * BLOC : (bs + 1) * BLOC, cs * 512 : (cs + 1) * 512] = sp
    if run_kwargs:
        return full, res
    return full
